# revision 24
# baseline (speedup 1.0000x reference)
"""GAT fusion kernel — nn_GAT_Fusion_2147483648587 (Trainium2 / Bass).

Takes FULL unsharded inputs, returns FULL [N] float32 output.

Strategy (node/graph parallel, per the sharding hint):
 - Host: append self-loops, sort nodes by in-degree, tile the sorted node
   list into windows of 128 dst nodes, deal windows round-robin across the
   8 cores (load balance + identical static loop bounds per core).  Edges
   are laid out dst-major: slot (p, j) of a window holds the j-th in-edge
   of the window's p-th dst node, so segment-softmax/aggregation become
   free-dim reductions with the dst on partitions.
 - Attention vectors fold into the weights: as = x @ (W1_h @ a_src_h), so
   the per-node table rows are [h | as] and "as" rides along the h gather.
 - Device: phase0 dense x@W1aug per shard -> AllGather bf16 [h|as] table;
   phase1 per window: one indirect-DMA row gather by src, exp(leakyrelu)
   edge logits, masked free-dim reduce for denominator and message sum,
   ELU, fused h2aug epilogue -> AllGather L2 table; phase2 per window: L2
   gather + aggregation + fused 2-layer MLP over [txt | g], streamed
   transposed txt, output accumulated in SBUF.
"""

import numpy as np
import ml_dtypes

BF16 = ml_dtypes.bfloat16

N_REAL = 50000
IN_DIM = 512
TXT_DIM = 768
NCORE = 8
P = 128
HEADS = 4
C1 = 32
F1 = HEADS * C1          # 128
F2 = 32
ZDIM = TXT_DIM + F2      # 800
HID = 64
NEG = 0.2

LAST_INFO = {}


# ----------------------------------------------------------------- host prep

def _host_graph(edge_index, n_real, ncore=NCORE):
    """Degree-sorted window partition + per-core slot arrays."""
    E = edge_index.shape[1]
    src = np.concatenate([edge_index[0], np.arange(n_real)]).astype(np.int64)
    dst = np.concatenate([edge_index[1], np.arange(n_real)]).astype(np.int64)

    SH = -(-n_real // (ncore * P)) * P      # nodes per core, multiple of P
    NT = SH * ncore
    W = NT // P
    WPC = W // ncore

    deg = np.bincount(dst, minlength=NT)
    sortperm = np.argsort(-deg, kind="stable")        # s -> orig node
    rank = np.empty(NT, np.int64)
    rank[sortperm] = np.arange(NT)

    # s -> global table row id g (c-major allgather layout)
    s_all = np.arange(NT)
    w_all = s_all // P
    g_of_s = (w_all % ncore) * SH + (w_all // ncore) * P + (s_all % P)

    degs = deg[sortperm]
    Jw = degs[0::P]                                   # window max degree
    Jbar = Jw[0::ncore].astype(np.int64)              # per-core k-th bound
    Jbar = np.maximum(Jbar, 1)
    SJ = int(Jbar.sum())
    colstart = np.zeros(WPC, np.int64)
    colstart[1:] = np.cumsum(Jbar)[:-1]

    s_dst = rank[dst]
    s_src = rank[src]
    w_e = s_dst // P
    p_e = (s_dst % P).astype(np.int64)
    c_e = (w_e % ncore).astype(np.int64)
    k_e = (w_e // ncore).astype(np.int64)
    g_src = g_of_s[s_src]

    order = np.argsort(s_dst, kind="stable")
    sd_sorted = s_dst[order]
    first = np.searchsorted(sd_sorted, sd_sorted, side="left")
    j_sorted = np.arange(len(order)) - first

    col = colstart[k_e[order]] + j_sorted
    row = p_e[order]
    core = c_e[order]
    val = g_src[order].astype(np.int32)

    offs = np.zeros((ncore, P, SJ), np.int32)         # pad -> row 0 (valid)
    mask = np.zeros((ncore, P, SJ), BF16)
    for c in range(ncore):
        sel = core == c
        offs[c, row[sel], col[sel]] = val[sel]
        mask[c, row[sel], col[sel]] = 1.0

    # per-core original-node lists, layout [P, WPC] matching device output
    k_idx = np.arange(WPC)
    p_idx = np.arange(P)
    nodes = np.empty((ncore, P, WPC), np.int64)
    for c in range(ncore):
        ss = (k_idx[None, :] * ncore + c) * P + p_idx[:, None]
        nodes[c] = sortperm[ss]

    return dict(SH=SH, NT=NT, WPC=WPC, Jbar=Jbar, SJ=SJ,
                colstart=colstart, offs=offs, mask=mask, nodes=nodes)


def _host_weights(W1, a_src1, a_dst1, b1, W2, a_src2, a_dst2, b2,
                  Wf1, bf1, Wf2, bf2):
    W1 = np.asarray(W1, np.float32)
    w_as1 = np.stack([W1[:, h * C1:(h + 1) * C1] @ np.asarray(a_src1, np.float32)[h]
                      for h in range(HEADS)], axis=1)       # [512, 4]
    w_ad1 = np.stack([W1[:, h * C1:(h + 1) * C1] @ np.asarray(a_dst1, np.float32)[h]
                      for h in range(HEADS)], axis=1)
    w1aug = np.concatenate([W1, w_as1, w_ad1], axis=1).astype(BF16)  # [512,136]

    W2 = np.asarray(W2, np.float32)
    w_as2 = W2 @ np.asarray(a_src2, np.float32)[0]
    w_ad2 = W2 @ np.asarray(a_dst2, np.float32)[0]
    w2aug = np.concatenate([W2, w_as2[:, None], w_ad2[:, None]],
                           axis=1).astype(BF16)             # [128, 34]

    wf1 = np.asarray(Wf1, np.float32).astype(BF16)          # [800, 64]
    wf2 = np.asarray(Wf2, np.float32).astype(BF16)          # [64, 1]

    b1r = np.tile(np.asarray(b1, np.float32)[None, :], (P, 1))     # [128,128]
    b2r = np.tile(np.asarray(b2, np.float32)[None, :], (P, 1))     # [128,32]
    bf1r = np.tile(np.asarray(bf1, np.float32)[None, :], (P, 1))   # [128,64]
    bf2r = np.tile(np.asarray(bf2, np.float32).reshape(1, 1), (P, 1))
    return w1aug, w2aug, wf1, wf2, b1r, b2r, bf1r, bf2r


# ------------------------------------------------------------ device program

def _make_tc_class(tile, mybir):
    """TileContext that hoists excess per-instruction sem waits into
    standalone EventSemaphore instructions (walrus encodes only a limited
    number of sync waits per instruction struct)."""
    from concourse.vector_clock import ScopedClock

    class HoistTC(tile.TileContext):
        WAIT_CAP = 1   # every walrus struct here encodes only 1 sync wait

        def _wait_cap(self, inst):
            if isinstance(inst, mybir.InstDMACopy | mybir.InstCollectiveCompute):
                return 1
            return self.WAIT_CAP

        def _add_instruction(self, inst):
            si = getattr(inst, "sync_info", None)
            cap = self._wait_cap(inst)
            if (
                si is not None
                and si.on_wait
                and len(si.on_wait) > cap
                and not isinstance(inst, mybir.InstEventSemaphore)
                and inst.engine != mybir.EngineType.Unassigned
            ):
                waits = list(si.on_wait)
                if isinstance(inst, mybir.InstDMACopy):
                    # Keep a DMA-completion wait in the descriptor (the DGE
                    # evaluates it without stalling the issuing engine);
                    # hoist engine-sem waits instead.
                    waits.sort(key=lambda w: 0 if "DMA" in w.ant_name else 1)
                keep = waits[:cap]
                hoist = waits[cap:]
                for i, w in enumerate(hoist):
                    ev = mybir.InstEventSemaphore(
                        name=f"{inst.name}_hw{i}", ins=[], outs=[])
                    ev.engine = inst.engine
                    ev.sync_info = mybir.SyncInfo(on_wait=[w], on_update=[])
                    super()._add_instruction(ev)
                inst.sync_info = mybir.SyncInfo(
                    on_wait=keep, on_update=list(si.on_update))
            super()._add_instruction(inst)

        def _drain_and_barrier(self, tick_clock, wait_clock):
            nopi = self.nc.sync.nop()
            wait_clock.add_sem_waits(
                nopi.ins, ScopedClock({None: tick_clock.global_clock}))
            raw = nopi.ins
            si = raw.sync_info
            waits = list(si.on_wait) if si else []
            raw.sync_info = mybir.SyncInfo(on_wait=waits[:1], on_update=[])
            handles = {}
            for v in self.sems.allocated().values():
                handles[getattr(v, "name", "")] = v
            for w in waits[1:]:
                h = handles.get(w.ant_name)
                if h is not None:
                    self.nc.sync.wait_ge(h, w.wait_value)
            self.nc.sync.drain()
            self.nc.all_engine_barrier()
            popped = self.nc._tile_sem_poison_stack.pop()
            assert popped is self._sem_poison
            self.nc.clear_and_free_semaphores(
                list(self.sems.allocated().values()))
            self.nc.all_engine_barrier()

    return HoistTC


def _build_program(geo):
    import concourse.bass as bass
    import concourse.mybir as mybir
    import concourse.tile as tile
    from concourse.masks import make_identity

    SH, NT, WPC, SJ = geo["SH"], geo["NT"], geo["WPC"], geo["SJ"]
    Jbar, colstart = geo["Jbar"], geo["colstart"]
    JMAX = int(Jbar.max())
    TW1 = F1 + 2 * HEADS          # 136 phase0 out cols
    T1 = F1 + HEADS               # 132 table1 row
    T2 = F2 + 1                   # 33 table2 row
    KQ = IN_DIM // P              # 4 k-tiles for x
    TQ = TXT_DIM // P             # 6 k-tiles for txt
    f32 = mybir.dt.float32
    bf16 = mybir.dt.bfloat16
    i32 = mybir.dt.int32
    AL = mybir.AluOpType
    ACT = mybir.ActivationFunctionType

    nc = bass.Bass("TRN2", target_bir_lowering=False, debug=False,
                   num_devices=NCORE, num_swdge_queues=4)

    # ---- dram params
    xT = nc.dram_tensor("xT", [IN_DIM, SH], bf16, kind="ExternalInput").ap()
    txtT = nc.dram_tensor("txtT", [TXT_DIM, SH], bf16, kind="ExternalInput").ap()
    offs_d = nc.dram_tensor("offs", [P, SJ], i32, kind="ExternalInput").ap()
    mask_d = nc.dram_tensor("mask", [P, SJ], bf16, kind="ExternalInput").ap()
    w1_d = nc.dram_tensor("w1aug", [IN_DIM, TW1], bf16, kind="ExternalInput").ap()
    w2_d = nc.dram_tensor("w2aug", [F1, T2 + 1], bf16, kind="ExternalInput").ap()
    wf1_d = nc.dram_tensor("wf1", [ZDIM, HID], bf16, kind="ExternalInput").ap()
    wf2_d = nc.dram_tensor("wf2", [HID, 1], bf16, kind="ExternalInput").ap()
    b1_d = nc.dram_tensor("b1r", [P, F1], f32, kind="ExternalInput").ap()
    b2_d = nc.dram_tensor("b2r", [P, F2], f32, kind="ExternalInput").ap()
    bf1_d = nc.dram_tensor("bf1r", [P, HID], f32, kind="ExternalInput").ap()
    bf2_d = nc.dram_tensor("bf2r", [P, 1], f32, kind="ExternalInput").ap()
    out_d = nc.dram_tensor("out", [P, WPC], f32, kind="ExternalOutput").ap()

    h1s = nc.dram_tensor("h1s", [SH, T1], bf16).ap()
    h1t = nc.dram_tensor("h1t", [NT, T1], bf16, addr_space="Shared").ap()
    h2s = nc.dram_tensor("h2s", [SH, T2], bf16).ap()
    h2t = nc.dram_tensor("h2t", [NT, T2], bf16, addr_space="Shared").ap()

    groups = [list(range(NCORE))]

    HoistTC = _make_tc_class(tile, mybir)
    with HoistTC(nc) as tc:
        import contextlib
        ctx = contextlib.ExitStack()
        with ctx:
            const = ctx.enter_context(tc.tile_pool(name="const", bufs=1))

            # resident constants
            w1_sb = const.tile([P, KQ, TW1], bf16, name="w1_sb")
            nc.sync.dma_start(out=w1_sb[:], in_=w1_d.rearrange("(q p) n -> p q n", p=P))
            w2_sb = const.tile([P, T2 + 1], bf16, name="w2_sb")
            nc.sync.dma_start(out=w2_sb[:], in_=w2_d[:])
            wf1_sb = const.tile([P, TQ + 1, HID], bf16, name="wf1_sb")
            nc.sync.dma_start(out=wf1_sb[:, :TQ, :],
                              in_=wf1_d[:TXT_DIM].rearrange("(q p) n -> p q n", p=P))
            nc.sync.dma_start(out=wf1_sb[:F2, TQ, :], in_=wf1_d[TXT_DIM:, :])
            wf2_sb = const.tile([HID, 1], bf16, name="wf2_sb")
            nc.sync.dma_start(out=wf2_sb[:], in_=wf2_d[:])
            b1_sb = const.tile([P, F1], f32, name="b1_sb")
            nc.sync.dma_start(out=b1_sb[:], in_=b1_d[:])
            b2_sb = const.tile([P, F2], f32, name="b2_sb")
            nc.sync.dma_start(out=b2_sb[:], in_=b2_d[:])
            bf1_sb = const.tile([P, HID], f32, name="bf1_sb")
            nc.sync.dma_start(out=bf1_sb[:], in_=bf1_d[:])
            bf2_sb = const.tile([P, 1], f32, name="bf2_sb")
            nc.sync.dma_start(out=bf2_sb[:], in_=bf2_d[:])
            offs_sb = const.tile([P, SJ], i32, name="offs_sb")
            nc.sync.dma_start(out=offs_sb[:], in_=offs_d[:])
            mask_sb = const.tile([P, SJ], bf16, name="mask_sb")
            nc.sync.dma_start(out=mask_sb[:], in_=mask_d[:])
            ident = const.tile([P, P], bf16, name="ident")
            make_identity(nc, ident[:])
            ad1_sb = const.tile([P, WPC, HEADS], f32, name="ad1_sb")
            ad2_sb = const.tile([P, WPC], f32, name="ad2_sb")
            outacc = const.tile([P, WPC], f32, name="outacc")

            # ---------------- phase 0: h1aug = x @ W1aug
            with tc.tile_pool(name="p0", bufs=2) as p0, \
                 tc.tile_pool(name="p0c", bufs=1) as p0c, \
                 tc.tile_pool(name="p0ps", bufs=2, space="PSUM") as p0ps:
                xT_sb = p0c.tile([P, KQ, SH], bf16, name="xT_sb")
                nc.sync.dma_start(out=xT_sb[:],
                                  in_=xT.rearrange("(q p) n -> p q n", p=P))
                for k in range(WPC):
                    ks = slice(k * P, (k + 1) * P)
                    hps = p0ps.tile([P, TW1], f32, name="hps")
                    for q in range(KQ):
                        nc.tensor.matmul(out=hps[:], lhsT=xT_sb[:, q, ks],
                                         rhs=w1_sb[:, q, :],
                                         start=(q == 0), stop=(q == KQ - 1))
                    h1a = p0.tile([P, T1], bf16, name="h1a")
                    nc.vector.tensor_copy(out=h1a[:], in_=hps[:, :T1])
                    nc.vector.tensor_copy(out=ad1_sb[:, k, :], in_=hps[:, T1:TW1])
                    nc.sync.dma_start(out=h1s[ks, :], in_=h1a[:])

            # allgather table 1
            nc.gpsimd.collective_compute(
                "AllGather", mybir.AluOpType.bypass, replica_groups=groups,
                ins=[h1s[:]], outs=[h1t[:]])

            # ---------------- phase 1: L1 edge aggregation + h2aug epilogue
            with tc.tile_pool(name="p1", bufs=2) as p1, \
                 tc.tile_pool(name="p1s", bufs=2) as p1s, \
                 tc.tile_pool(name="p1ps", bufs=2, space="PSUM") as p1ps:
                for k in range(WPC):
                    J = int(Jbar[k])
                    cs = int(colstart[k])
                    ks = slice(k * P, (k + 1) * P)
                    gbuf = p1.tile([P, JMAX, T1], bf16, name="gbuf", bufs=3)
                    for j in range(J):
                        nc.gpsimd.indirect_dma_start(
                            out=gbuf[:, j, :], out_offset=None,
                            in_=h1t[:],
                            in_offset=bass.IndirectOffsetOnAxis(
                                ap=offs_sb[:, cs + j:cs + j + 1], axis=0))
                    # e = exp(lrelu(as + ad)) * mask
                    epre = p1s.tile([P, JMAX, HEADS], f32, name="epre")
                    nc.vector.tensor_tensor(
                        out=epre[:, :J, :], in0=gbuf[:, :J, F1:T1],
                        in1=ad1_sb[:, k, :][:, None, :].to_broadcast([P, J, HEADS]),
                        op=AL.add)
                    elr = p1s.tile([P, JMAX, HEADS], f32, name="elr")
                    nc.vector.scalar_tensor_tensor(
                        out=elr[:, :J, :], in0=epre[:, :J, :], scalar=NEG,
                        in1=epre[:, :J, :], op0=AL.mult, op1=AL.max)
                    expv = p1s.tile([P, JMAX, HEADS], bf16, name="expv")
                    nc.scalar.activation(out=expv[:, :J, :], in_=elr[:, :J, :],
                                         func=ACT.Exp)
                    expm = p1s.tile([P, JMAX, HEADS], bf16, name="expm")
                    nc.vector.tensor_tensor(
                        out=expm[:, :J, :], in0=expv[:, :J, :],
                        in1=mask_sb[:, cs:cs + J][:, :, None].to_broadcast(
                            [P, J, HEADS]),
                        op=AL.mult)
                    den = p1s.tile([P, HEADS], f32, name="den")
                    nc.vector.tensor_reduce(
                        out=den[:], in_=expm[:, :J, :].transpose([0, 2, 1]),
                        axis=mybir.AxisListType.X, op=AL.add)
                    nc.vector.tensor_scalar_add(den[:], den[:], 1e-16)
                    rden = p1s.tile([P, HEADS], f32, name="rden")
                    nc.vector.reciprocal(out=rden[:], in_=den[:])
                    # messages
                    msg = p1.tile([P, JMAX, F1], bf16, name="msg")
                    nc.vector.tensor_tensor(
                        out=msg[:, :J, :].rearrange("p j (h c) -> p j h c", h=HEADS),
                        in0=gbuf[:, :J, :F1].rearrange("p j (h c) -> p j h c", h=HEADS),
                        in1=expm[:, :J, :, None].to_broadcast([P, J, HEADS, C1]),
                        op=AL.mult)
                    red = p1s.tile([P, F1], f32, name="red")
                    nc.vector.tensor_reduce(
                        out=red[:], in_=msg[:, :J, :].transpose([0, 2, 1]),
                        axis=mybir.AxisListType.X, op=AL.add)
                    # normalize + bias + ELU
                    h1e = p1s.tile([P, F1], f32, name="h1e")
                    nc.vector.tensor_tensor(
                        out=h1e[:].rearrange("p (h c) -> p h c", h=HEADS),
                        in0=red[:].rearrange("p (h c) -> p h c", h=HEADS),
                        in1=rden[:, :, None].to_broadcast([P, HEADS, C1]),
                        op=AL.mult)
                    nc.vector.tensor_add(h1e[:], h1e[:], b1_sb[:])
                    mn = p1s.tile([P, F1], f32, name="mn")
                    nc.vector.tensor_scalar_min(mn[:], h1e[:], 0.0)
                    ex = p1s.tile([P, F1], f32, name="ex")
                    nc.scalar.activation(out=ex[:], in_=mn[:], func=ACT.Exp)
                    rl = p1s.tile([P, F1], f32, name="rl")
                    nc.vector.tensor_scalar_max(rl[:], h1e[:], 0.0)
                    h1eb = p1s.tile([P, F1], bf16, name="h1eb")
                    nc.vector.scalar_tensor_tensor(
                        out=h1eb[:], in0=ex[:], scalar=-1.0, in1=rl[:],
                        op0=AL.add, op1=AL.add)
                    # h2aug epilogue
                    tp = p1ps.tile([P, P], bf16, name="tp")
                    nc.tensor.transpose(out=tp[:], in_=h1eb[:], identity=ident[:])
                    h1eT = p1s.tile([P, P], bf16, name="h1eT")
                    nc.vector.tensor_copy(out=h1eT[:], in_=tp[:])
                    h2ps = p1ps.tile([P, T2 + 1], f32, name="h2ps")
                    nc.tensor.matmul(out=h2ps[:], lhsT=h1eT[:], rhs=w2_sb[:],
                                     start=True, stop=True)
                    h2a = p1s.tile([P, T2], bf16, name="h2a")
                    nc.vector.tensor_copy(out=h2a[:], in_=h2ps[:, :T2])
                    nc.vector.tensor_copy(out=ad2_sb[:, k:k + 1],
                                          in_=h2ps[:, T2:T2 + 1])
                    nc.sync.dma_start(out=h2s[ks, :], in_=h2a[:])

            # allgather table 2
            nc.gpsimd.collective_compute(
                "AllGather", mybir.AluOpType.bypass, replica_groups=groups,
                ins=[h2s[:]], outs=[h2t[:]])

            # ---------------- phase 2+3: L2 edge aggregation + fusion MLP
            with tc.tile_pool(name="p2", bufs=2) as p2, \
                 tc.tile_pool(name="p2c", bufs=1) as p2c, \
                 tc.tile_pool(name="p2s", bufs=2) as p2s, \
                 tc.tile_pool(name="p2ps", bufs=2, space="PSUM") as p2ps:
                txtT_sb = p2c.tile([P, TQ, SH], bf16, name="txtT_sb")
                nc.gpsimd.dma_start(out=txtT_sb[:],
                                    in_=txtT.rearrange("(q p) n -> p q n", p=P))
                for k in range(WPC):
                    J = int(Jbar[k])
                    cs = int(colstart[k])
                    ks = slice(k * P, (k + 1) * P)
                    g2 = p2.tile([P, JMAX, T2], bf16, name="g2", bufs=3)
                    for j in range(J):
                        nc.gpsimd.indirect_dma_start(
                            out=g2[:, j, :], out_offset=None,
                            in_=h2t[:],
                            in_offset=bass.IndirectOffsetOnAxis(
                                ap=offs_sb[:, cs + j:cs + j + 1], axis=0))
                    epre2 = p2s.tile([P, JMAX], f32, name="epre2")
                    nc.vector.tensor_tensor(
                        out=epre2[:, :J], in0=g2[:, :J, F2],
                        in1=ad2_sb[:, k:k + 1].to_broadcast([P, J]),
                        op=AL.add)
                    elr2 = p2s.tile([P, JMAX], f32, name="elr2")
                    nc.vector.scalar_tensor_tensor(
                        out=elr2[:, :J], in0=epre2[:, :J], scalar=NEG,
                        in1=epre2[:, :J], op0=AL.mult, op1=AL.max)
                    expv2 = p2s.tile([P, JMAX], bf16, name="expv2")
                    nc.scalar.activation(out=expv2[:, :J], in_=elr2[:, :J],
                                         func=ACT.Exp)
                    expm2 = p2s.tile([P, JMAX], bf16, name="expm2")
                    nc.vector.tensor_tensor(
                        out=expm2[:, :J], in0=expv2[:, :J],
                        in1=mask_sb[:, cs:cs + J], op=AL.mult)
                    den2 = p2s.tile([P, 1], f32, name="den2")
                    nc.vector.tensor_reduce(
                        out=den2[:], in_=expm2[:, :J],
                        axis=mybir.AxisListType.X, op=AL.add)
                    nc.vector.tensor_scalar_add(den2[:], den2[:], 1e-16)
                    rden2 = p2s.tile([P, 1], f32, name="rden2")
                    nc.vector.reciprocal(out=rden2[:], in_=den2[:])
                    msg2 = p2.tile([P, JMAX, F2], bf16, name="msg2")
                    nc.vector.tensor_tensor(
                        out=msg2[:, :J, :], in0=g2[:, :J, :F2],
                        in1=expm2[:, :J, None].to_broadcast([P, J, F2]),
                        op=AL.mult)
                    red2 = p2s.tile([P, F2], f32, name="red2")
                    nc.vector.tensor_reduce(
                        out=red2[:], in_=msg2[:, :J, :].transpose([0, 2, 1]),
                        axis=mybir.AxisListType.X, op=AL.add)
                    gsb = p2s.tile([P, F2], f32, name="gsb")
                    nc.vector.tensor_tensor(
                        out=gsb[:], in0=red2[:],
                        in1=rden2[:].to_broadcast([P, F2]), op=AL.mult)
                    nc.vector.tensor_add(gsb[:], gsb[:], b2_sb[:])
                    gb = p2s.tile([P, F2], bf16, name="gb")
                    nc.vector.tensor_copy(out=gb[:], in_=gsb[:])
                    # fusion MLP
                    gT_ps = p2ps.tile([F2, P], bf16, name="gT_ps")
                    nc.tensor.transpose(out=gT_ps[:], in_=gb[:], identity=ident[:])
                    gT = p2s.tile([F2, P], bf16, name="gT")
                    nc.vector.tensor_copy(out=gT[:], in_=gT_ps[:])
                    z1ps = p2ps.tile([P, HID], f32, name="z1ps")
                    for q in range(TQ):
                        nc.tensor.matmul(out=z1ps[:], lhsT=txtT_sb[:, q, ks],
                                         rhs=wf1_sb[:, q, :],
                                         start=(q == 0), stop=False)
                    nc.tensor.matmul(out=z1ps[:], lhsT=gT[:], rhs=wf1_sb[:F2, TQ, :],
                                     start=False, stop=True)
                    z1 = p2s.tile([P, HID], f32, name="z1")
                    nc.vector.tensor_add(z1[:], z1ps[:], bf1_sb[:])
                    z1b = p2s.tile([P, HID], bf16, name="z1b")
                    nc.vector.tensor_scalar_max(z1b[:], z1[:], 0.0)
                    z1T_ps = p2ps.tile([HID, P], bf16, name="z1T_ps")
                    nc.tensor.transpose(out=z1T_ps[:], in_=z1b[:], identity=ident[:])
                    z1T = p2s.tile([HID, P], bf16, name="z1T")
                    nc.vector.tensor_copy(out=z1T[:], in_=z1T_ps[:])
                    ops = p2ps.tile([P, 1], f32, name="ops")
                    nc.tensor.matmul(out=ops[:], lhsT=z1T[:], rhs=wf2_sb[:],
                                     start=True, stop=True)
                    nc.vector.tensor_add(outacc[:, k:k + 1], ops[:], bf2_sb[:])

            nc.sync.dma_start(out=out_d[:], in_=outacc[:])

    return nc


# ------------------------------------------------------------------- driver

def kernel(txt, x, W1, a_src1, a_dst1, b1, W2, a_src2, a_dst2, b2,
           Wf1, bf1, Wf2, bf2, edge_index):
    import time
    from concourse.bass_utils import run_bass_kernel_spmd

    t0 = time.perf_counter()
    x = np.asarray(x, np.float32)
    txt = np.asarray(txt, np.float32)
    edge_index = np.asarray(edge_index, np.int64)

    geo = _host_graph(edge_index, N_REAL)
    SH, NT, WPC = geo["SH"], geo["NT"], geo["WPC"]
    nodes = geo["nodes"]

    w1aug, w2aug, wf1, wf2, b1r, b2r, bf1r, bf2r = _host_weights(
        W1, a_src1, a_dst1, b1, W2, a_src2, a_dst2, b2, Wf1, bf1, Wf2, bf2)

    xp = np.zeros((NT, IN_DIM), np.float32)
    xp[:N_REAL] = x
    txtp = np.zeros((NT, TXT_DIM), np.float32)
    txtp[:N_REAL] = txt

    in_maps = []
    for c in range(NCORE):
        nl = nodes[c].reshape(P, WPC)          # [P, WPC] original ids
        order = nl.T.reshape(-1)               # row r = k*P+p -> node
        xT_c = np.ascontiguousarray(xp[order].T.astype(BF16))
        txtT_c = np.ascontiguousarray(txtp[order].T.astype(BF16))
        in_maps.append(dict(
            xT=xT_c, txtT=txtT_c,
            offs=geo["offs"][c], mask=geo["mask"][c],
            w1aug=w1aug, w2aug=w2aug, wf1=wf1, wf2=wf2,
            b1r=b1r, b2r=b2r, bf1r=bf1r, bf2r=bf2r))

    t1 = time.perf_counter()
    nc = _build_program(geo)
    t2 = time.perf_counter()

    br = run_bass_kernel_spmd(nc, in_maps, list(range(NCORE)))
    t3 = time.perf_counter()

    out = np.empty(NT, np.float32)
    for c in range(NCORE):
        out[nodes[c].reshape(-1)] = np.asarray(
            br.results[c]["out"], np.float32).reshape(-1)
    res = out[:N_REAL]

    LAST_INFO.update(dict(
        host_prep_s=t1 - t0, build_s=t2 - t1, run_s=t3 - t2,
        exec_time_ns=br.exec_time_ns))
    return res


# ------------------------------------------------- timing (test.py helper)

def _make_runner(nc, in_maps):
    """Reusable jitted runner for one Bass program (mirrors
    bass2jax.run_bass_via_pjrt, minus donation so inputs stay resident)."""
    import jax
    import numpy as np
    from jax.sharding import Mesh, PartitionSpec
    from jax.experimental.shard_map import shard_map
    from concourse import bass2jax, mybir

    bass2jax.install_neuronx_cc_hook()
    n_cores = len(in_maps)
    in_names, out_names, out_avals, zero_outs = [], [], [], []
    partition_name = (nc.partition_id_tensor.name
                      if nc.partition_id_tensor else None)
    for alloc in nc.m.functions[0].allocations:
        if not isinstance(alloc, mybir.MemoryLocationSet):
            continue
        name = alloc.memorylocations[0].name
        if alloc.kind == "ExternalInput":
            if name != partition_name:
                in_names.append(name)
        elif alloc.kind == "ExternalOutput":
            out_names.append(name)
            shape = tuple(alloc.tensor_shape)
            dtype = mybir.dt.np(alloc.dtype)
            out_avals.append(jax.core.ShapedArray(shape, dtype))
            zero_outs.append(np.zeros(shape, dtype))
    n_params = len(in_names)
    all_in = list(in_names) + out_names
    if partition_name is not None:
        all_in.append(partition_name)

    def _body(*args):
        operands = list(args)
        if partition_name is not None:
            operands.append(bass2jax.partition_id_tensor())
        outs = bass2jax._bass_exec_p.bind(
            *operands, out_avals=tuple(out_avals), in_names=tuple(all_in),
            out_names=tuple(out_names), lowering_input_output_aliases=(),
            sim_require_finite=True, sim_require_nnan=True, nc=nc)
        return tuple(outs)

    devices = jax.devices()[:n_cores]
    mesh = Mesh(np.asarray(devices), ("core",))
    in_specs = (PartitionSpec("core"),) * (n_params + len(out_names))
    out_specs = (PartitionSpec("core"),) * len(out_names)
    jf = jax.jit(shard_map(_body, mesh=mesh, in_specs=in_specs,
                           out_specs=out_specs, check_rep=False),
                 keep_unused=True)
    concat_in = [np.concatenate([np.asarray(in_maps[c][n])
                                 for c in range(n_cores)], axis=0)
                 for n in in_names]
    concat_zeros = [np.zeros((n_cores * z.shape[0], *z.shape[1:]), z.dtype)
                    for z in zero_outs]
    sh = jax.sharding.NamedSharding(mesh, PartitionSpec("core"))
    dev_args = [jax.device_put(a, sh) for a in concat_in + concat_zeros]
    return jf, dev_args


def bench_exec_ns(txt, x, W1, a_src1, a_dst1, b1, W2, a_src2, a_dst2, b2,
                  Wf1, bf1, Wf2, bf2, edge_index, reps=8):
    """Estimate device execution time: (full program) - (null program),
    both timed with a reused jitted executable and resident inputs."""
    import time
    import jax

    x = np.asarray(x, np.float32)
    txt = np.asarray(txt, np.float32)
    edge_index = np.asarray(edge_index, np.int64)
    geo = _host_graph(edge_index, N_REAL)
    SH, NT, WPC = geo["SH"], geo["NT"], geo["WPC"]
    nodes = geo["nodes"]
    w1aug, w2aug, wf1, wf2, b1r, b2r, bf1r, bf2r = _host_weights(
        W1, a_src1, a_dst1, b1, W2, a_src2, a_dst2, b2, Wf1, bf1, Wf2, bf2)
    xp = np.zeros((NT, IN_DIM), np.float32)
    xp[:N_REAL] = x
    txtp = np.zeros((NT, TXT_DIM), np.float32)
    txtp[:N_REAL] = txt
    in_maps = []
    for c in range(NCORE):
        nl = nodes[c].reshape(P, WPC)
        order = nl.T.reshape(-1)
        in_maps.append(dict(
            xT=np.ascontiguousarray(xp[order].T.astype(BF16)),
            txtT=np.ascontiguousarray(txtp[order].T.astype(BF16)),
            offs=geo["offs"][c], mask=geo["mask"][c],
            w1aug=w1aug, w2aug=w2aug, wf1=wf1, wf2=wf2,
            b1r=b1r, b2r=b2r, bf1r=bf1r, bf2r=bf2r))

    def _null_program():
        import concourse.bass as bass
        import concourse.mybir as mybir
        import concourse.tile as tile
        f32 = mybir.dt.float32
        nc = bass.Bass("TRN2", target_bir_lowering=False, debug=False,
                       num_devices=NCORE, num_swdge_queues=4)
        for nm, shp, dt in (
                ("xT", [IN_DIM, SH], mybir.dt.bfloat16),
                ("txtT", [TXT_DIM, SH], mybir.dt.bfloat16),
                ("offs", [P, geo["SJ"]], mybir.dt.int32),
                ("mask", [P, geo["SJ"]], mybir.dt.bfloat16),
                ("w1aug", [IN_DIM, 136], mybir.dt.bfloat16),
                ("w2aug", [F1, 34], mybir.dt.bfloat16),
                ("wf1", [ZDIM, HID], mybir.dt.bfloat16),
                ("wf2", [HID, 1], mybir.dt.bfloat16),
                ("b1r", [P, F1], f32), ("b2r", [P, F2], f32),
                ("bf1r", [P, HID], f32), ("bf2r", [P, 1], f32)):
            nc.dram_tensor(nm, shp, dt, kind="ExternalInput")
        out_d = nc.dram_tensor("out", [P, WPC], f32, kind="ExternalOutput").ap()
        HoistTC = _make_tc_class(tile, mybir)
        with HoistTC(nc) as tc:
            with tc.tile_pool(name="z", bufs=1) as z:
                t = z.tile([P, WPC], f32, name="t")
                nc.vector.memset(t[:], 0.0)
                nc.sync.dma_start(out=out_d[:], in_=t[:])
        return nc

    jf_n, args_n = _make_runner(_null_program(), in_maps)
    jf_f, args_f = _make_runner(_build_program(geo), in_maps)

    def _block(jf, args):
        # back-to-back same-program calls: first call pays the NEFF switch,
        # later calls are steady-state
        ts = []
        for _ in range(reps):
            t0 = time.perf_counter()
            jax.block_until_ready(jf(*args))
            ts.append(time.perf_counter() - t0)
        return min(ts)

    _block(jf_n, args_n)          # tunnel warm-up
    _block(jf_f, args_f)
    t_null = _block(jf_n, args_n)
    t_full = _block(jf_f, args_f)
    return max(int((t_full - t_null) * 1e9), 0), t_full, t_null
